# revision 1
# baseline (speedup 1.0000x reference)
"""2-layer GCN (GCNConv -> ReLU -> GCNConv -> log_softmax) on 8 Trainium2 cores.

Sharding: nodes are relabeled (sorted by in-degree) and dealt out in blocks of
128 round-robin across the 8 cores (graph/data parallel).  Each core:
  - transforms its node slice (x @ W1) on the tensor engine,
  - AllGathers the dinv-scaled hidden table,
  - aggregates messages for its nodes (indirect-DMA row gathers + vector-
    engine weighted reduction); self-loop terms are kept in SBUF and never
    gathered; dinv[col] is folded into the edge weights host-side,
  - repeats for layer 2 (aggregate-then-transform, which commutes),
  - finishes with W2 transform + a batched log_softmax (single Exp/Ln
    activation-table load each).
Weight matrices are replicated; outputs are gathered/unpermuted on the host.
"""

import ml_dtypes
import numpy as np

N_NODES = 100000
N_EDGES = 1600000
F_IN = 500
F_PAD = 512
H = 64
C = 16
N_CORES = 8
P = 128
NPAD = 100352          # 8 * 12544 node slots after padding
ND = NPAD - N_NODES    # dummy slots (placed first: lowest degree)
NB = NPAD // (P * N_CORES)   # 98 local blocks per core
NLOC = NB * P                # 12544 local node slots per core

_PROG_CACHE = {}
PROFILE = False
LAST_EXEC_NS = None
LAST_TRACE = None
LAST_PROFILE_JSON = None


def _build_program(T, d_loc, coffs, dmax):
    import concourse.bacc as bacc
    import concourse.bass as bass
    import concourse.mybir as mybir
    from concourse.tile import TileContext
    from concourse.masks import make_identity
    from contextlib import ExitStack

    dt = mybir.dt.float32
    bf = mybir.dt.bfloat16
    nc = bacc.Bacc("TRN2", target_bir_lowering=False, debug=False,
                   num_devices=N_CORES, dynamic_dma_scratch_size=32768)

    xT = nc.dram_tensor("xT", [F_PAD, NLOC], bf, kind="ExternalInput")
    W1p = nc.dram_tensor("W1p", [F_PAD, H], bf, kind="ExternalInput")
    b1t = nc.dram_tensor("b1t", [P, H], dt, kind="ExternalInput")
    W2t = nc.dram_tensor("W2t", [H, C], dt, kind="ExternalInput")
    b2t = nc.dram_tensor("b2t", [P, C], dt, kind="ExternalInput")
    wsl = nc.dram_tensor("wsl", [P, T], bf, kind="ExternalInput")
    isl = nc.dram_tensor("isl", [P, T], mybir.dt.int32, kind="ExternalInput")
    dvt = nc.dram_tensor("dvt", [P, NB], dt, kind="ExternalInput")
    outd = nc.dram_tensor("outd", [NLOC, C], dt, kind="ExternalOutput")

    # gather tables: dedicated DRAM tensors (offset-0 requirement of
    # indirect_dma_start sources)
    hs1_loc = nc.dram_tensor("hs1_loc", [NLOC, H], bf)
    hs1_full = nc.dram_tensor("hs1_full", [NPAD, H], bf, addr_space="Shared")
    g1_loc = nc.dram_tensor("g1_loc", [NLOC, H], bf)
    g1_full = nc.dram_tensor("g1_full", [NPAD, H], bf, addr_space="Shared")

    SG = 7            # blocks per transform supergroup
    NSG = NB // SG    # 14
    HB = NB // 2
    HROWS = N_CORES * HB * P

    def bcast_inner(ap, n):
        # append a step-0 inner dim of size n to an AP view
        return bass.AP(ap.tensor, ap.offset, list(ap.ap) + [[0, n]])

    def swap_last2(ap):
        a = list(ap.ap)
        a[-1], a[-2] = a[-2], a[-1]
        return bass.AP(ap.tensor, ap.offset, a)

    with TileContext(nc) as tc, ExitStack() as ctx:
        cp = ctx.enter_context(tc.tile_pool(name="const", bufs=1))
        xp = ctx.enter_context(tc.tile_pool(name="xsg", bufs=2))
        hp = ctx.enter_context(tc.tile_pool(name="hrow", bufs=4))
        gp = ctx.enter_context(tc.tile_pool(name="gsg", bufs=2))
        mp = ctx.enter_context(tc.tile_pool(name="msg", bufs=6))
        ap_ = ctx.enter_context(tc.tile_pool(name="agg", bufs=4))
        pp = ctx.enter_context(tc.tile_pool(name="ps", bufs=4, space="PSUM"))
        pt = ctx.enter_context(tc.tile_pool(name="pstr", bufs=2, space="PSUM"))
        po = ctx.enter_context(tc.tile_pool(name="pso", bufs=2, space="PSUM"))
        sp = ctx.enter_context(tc.tile_pool(name="small", bufs=8))

        # ---- constants ----
        w1t = cp.tile([P, 4, H], bf)
        for k in range(4):
            nc.sync.dma_start(w1t[:, k, :], W1p[k * P:(k + 1) * P, :])
        b1r = cp.tile([P, H], dt)
        nc.sync.dma_start(b1r[:], b1t[:])
        w2 = cp.tile([H, C], dt)
        nc.sync.dma_start(w2[:], W2t[:])
        b2r = cp.tile([P, C], dt)
        nc.sync.dma_start(b2r[:], b2t[:])
        wslt = cp.tile([P, T], bf)
        nc.sync.dma_start(wslt[:], wsl[:])
        islt = cp.tile([P, T], mybir.dt.int32)
        nc.sync.dma_start(islt[:], isl[:])
        dinv = cp.tile([P, NB], dt)
        nc.sync.dma_start(dinv[:], dvt[:])
        ident = cp.tile([P, P], dt)
        make_identity(nc, ident[:])
        dsq = cp.tile([P, NB], dt)
        nc.vector.tensor_tensor(out=dsq[:], in0=dinv[:], in1=dinv[:],
                                op=mybir.AluOpType.mult)

        # persistent per-block tiles
        hself = cp.tile([P, NB, H], dt)    # dinv^2 * t1 + b1 (self term + bias)
        g1self = cp.tile([P, NB, H], dt)   # dinv * g1 (layer-2 self term)
        o_all = cp.tile([P, NB, C], dt)
        maxv = cp.tile([P, NB], dt)
        s_all = cp.tile([P, NB], dt)
        lns = cp.tile([P, NB], dt)

        # ---- transform: t1 = x @ W1 ; hs1 = dinv * t1 ----
        for sg in range(NSG):
            c0 = sg * SG * P
            xk = xp.tile([P, 4, SG * P], bf, tag="xk")
            for k in range(4):
                nc.sync.dma_start(xk[:, k, :], xT[k * P:(k + 1) * P, c0:c0 + SG * P])
            hsg = gp.tile([P, SG, H], bf, tag="hsg")
            for bl in range(SG):
                b = sg * SG + bl
                ps = pp.tile([P, H], dt, tag="pst")
                for k in range(4):
                    nc.tensor.matmul(ps[:], lhsT=xk[:, k, bl * P:(bl + 1) * P],
                                     rhs=w1t[:, k, :],
                                     start=(k == 0), stop=(k == 3))
                nc.vector.tensor_scalar(hsg[:, bl, :], ps[:], dinv[:, b:b + 1],
                                        None, op0=mybir.AluOpType.mult)
                nc.vector.scalar_tensor_tensor(
                    out=hself[:, b, :], in0=ps[:],
                    scalar=dsq[:, b:b + 1], in1=b1r[:],
                    op0=mybir.AluOpType.mult, op1=mybir.AluOpType.add)
            for bl in range(SG):
                b = sg * SG + bl
                nc.sync.dma_start(hs1_loc[b * P:(b + 1) * P, :], hsg[:, bl, :])
            if (sg + 1) * SG == HB:
                # first half of the local slice is complete -> overlap its
                # AllGather with the rest of the transform
                nc.gpsimd.collective_compute(
                    "AllGather", mybir.AluOpType.bypass,
                    replica_groups=[list(range(N_CORES))],
                    ins=[hs1_loc[0:HB * P, :]], outs=[hs1_full[0:HROWS, :]])

        nc.gpsimd.collective_compute(
            "AllGather", mybir.AluOpType.bypass,
            replica_groups=[list(range(N_CORES))],
            ins=[hs1_loc[HB * P:NLOC, :]], outs=[hs1_full[HROWS:NPAD, :]])

        # ---- aggregation layers ----
        def agg_layer(table, post, blo, bhi):
            for b in range(blo, bhi):
                db = d_loc[b]
                msg = mp.tile([P, dmax, H], bf, tag="msg")
                for j in range(db):
                    nc.gpsimd.indirect_dma_start(
                        out=msg[:, j, :], out_offset=None, in_=table[:],
                        in_offset=bass.IndirectOffsetOnAxis(
                            ap=islt[:, coffs[b] + j:coffs[b] + j + 1], axis=0))
                # weighted sum over the db in-edge slots (weights include
                # dinv[col])
                wv = bcast_inner(wslt[:, coffs[b]:coffs[b] + db], H)
                nc.vector.tensor_tensor(out=msg[:, :db, :], in0=msg[:, :db, :],
                                        in1=wv, op=mybir.AluOpType.mult)
                agg = ap_.tile([P, H], dt, tag="agg")
                nc.vector.reduce_sum(agg[:], swap_last2(msg[:, :db, :]),
                                     axis=mybir.AxisListType.X)
                post(b, agg)

        # layer 1 post: aggf = agg + (dinv^2 t1 + b1) ; g1 = dinv * relu(aggf)
        g1sg = {}

        def post1(b, agg):
            nc.vector.tensor_tensor(out=agg[:], in0=agg[:],
                                    in1=hself[:, b, :],
                                    op=mybir.AluOpType.add)
            bl = b % SG
            if bl == 0:
                g1t_new = gp.tile([P, SG, H], bf, tag="g1sg")
                g1sg[0] = g1t_new
            g1t = g1sg[0]
            nc.vector.tensor_scalar_max(agg[:], agg[:], 0.0)
            nc.vector.tensor_scalar(g1t[:, bl, :], agg[:], dinv[:, b:b + 1],
                                    None, op0=mybir.AluOpType.mult)
            nc.vector.tensor_scalar(g1self[:, b, :], agg[:],
                                    dsq[:, b:b + 1], None,
                                    op0=mybir.AluOpType.mult)
            nc.sync.dma_start(g1_loc[b * P:(b + 1) * P, :], g1t[:, bl, :])

        agg_layer(hs1_full, post1, 0, HB)
        # first half of g1 done -> overlap its AllGather with the second half
        nc.gpsimd.collective_compute(
            "AllGather", mybir.AluOpType.bypass,
            replica_groups=[list(range(N_CORES))],
            ins=[g1_loc[0:HB * P, :]], outs=[g1_full[0:HROWS, :]])
        agg_layer(hs1_full, post1, HB, NB)
        nc.gpsimd.collective_compute(
            "AllGather", mybir.AluOpType.bypass,
            replica_groups=[list(range(N_CORES))],
            ins=[g1_loc[HB * P:NLOC, :]], outs=[g1_full[HROWS:NPAD, :]])

        # layer 2 post: sc2 = agg2 + dinv*g1 ; o = sc2 @ W2 + b2 (+ row max)
        def post2(b, agg):
            nc.vector.tensor_tensor(out=agg[:], in0=agg[:],
                                    in1=g1self[:, b, :],
                                    op=mybir.AluOpType.add)
            ptr = pt.tile([H, P], dt, tag="ptr")
            nc.tensor.transpose(ptr[:], agg[:], ident[:])
            scT = sp.tile([H, P], dt, tag="scT")
            nc.vector.tensor_copy(scT[:], ptr[:])
            pso = po.tile([P, C], dt, tag="pso")
            nc.tensor.matmul(pso[:], lhsT=scT[:], rhs=w2[:],
                             start=True, stop=True)
            nc.vector.tensor_tensor(out=o_all[:, b, :], in0=pso[:],
                                    in1=b2r[:], op=mybir.AluOpType.add)
            nc.vector.tensor_reduce(maxv[:, b:b + 1], o_all[:, b, :],
                                    axis=mybir.AxisListType.X,
                                    op=mybir.AluOpType.max)

        # ---- batched log_softmax tail (in place on o_all), per half so the
        # first half overlaps the second half's gathers ----
        def softmax_tail(blo, bhi):
            nc.vector.tensor_tensor(out=o_all[:, blo:bhi, :],
                                    in0=o_all[:, blo:bhi, :],
                                    in1=bcast_inner(maxv[:, blo:bhi], C),
                                    op=mybir.AluOpType.subtract)
            for b in range(blo, bhi):
                e = sp.tile([P, C], dt, tag="e")
                nc.scalar.activation(e[:], o_all[:, b, :],
                                     mybir.ActivationFunctionType.Exp,
                                     accum_out=s_all[:, b:b + 1])
            nc.scalar.activation(lns[:, blo:bhi], s_all[:, blo:bhi],
                                 mybir.ActivationFunctionType.Ln)
            nc.vector.tensor_tensor(out=o_all[:, blo:bhi, :],
                                    in0=o_all[:, blo:bhi, :],
                                    in1=bcast_inner(lns[:, blo:bhi], C),
                                    op=mybir.AluOpType.subtract)
            for b in range(blo, bhi):
                nc.sync.dma_start(outd[b * P:(b + 1) * P, :], o_all[:, b, :])

        for blo, bhi in ((0, HB), (HB, 86), (86, NB)):
            agg_layer(g1_full, post2, blo, bhi)
            softmax_tail(blo, bhi)

    nc.compile()
    return nc


def _prep(x, edge_index, edge_weight, W1, b1, W2, b2):
    x = np.asarray(x, dtype=np.float32)
    ei = np.asarray(edge_index).astype(np.int64)
    ew = np.asarray(edge_weight, dtype=np.float32)
    W1 = np.asarray(W1, dtype=np.float32)
    b1 = np.asarray(b1, dtype=np.float32)
    W2 = np.asarray(W2, dtype=np.float32)
    b2 = np.asarray(b2, dtype=np.float32)

    rows, cols, ws = ei[0], ei[1], ew   # real edges only; self loops special-cased

    # degrees include the self loop (weight 1), matching the reference
    indeg = np.bincount(cols, minlength=N_NODES)
    degw = np.bincount(cols, weights=ws.astype(np.float64),
                       minlength=N_NODES).astype(np.float32) + 1.0
    dinv_old = 1.0 / np.sqrt(degw)      # deg > 0 always (self loop)

    perm = np.argsort(indeg, kind="stable")          # old ids, ascending degree
    new_of_old = np.empty(N_NODES, dtype=np.int64)
    new_of_old[perm] = np.arange(N_NODES, dtype=np.int64) + ND

    HB = NB // 2
    HROWS = N_CORES * HB * P

    def table_row_of_new(s):
        kg = s // P
        p = s % P
        c = kg % N_CORES
        b = kg // N_CORES
        lo = c * HB * P + b * P + p
        hi = HROWS + c * (NB - HB) * P + (b - HB) * P + p
        return np.where(b < HB, lo, hi)

    r_new = new_of_old[rows]
    c_new = new_of_old[cols]
    kg = c_new // P
    core_of_edge = kg % N_CORES
    b_of_edge = kg // N_CORES
    p_of_edge = c_new % P
    src_row = table_row_of_new(r_new)

    # per-local-block chunk counts across cores (no self loops)
    cnt_key = ((core_of_edge * P + p_of_edge) * NB + b_of_edge)
    cnt = np.bincount(cnt_key, minlength=N_CORES * P * NB).reshape(
        N_CORES, P, NB)
    d_loc = cnt.max(axis=(0, 1)).astype(np.int64)
    d_loc = np.maximum(d_loc, 1)
    coffs = np.zeros(NB, dtype=np.int64)
    coffs[1:] = np.cumsum(d_loc)[:-1]
    T = int(d_loc.sum())
    dmax = int(d_loc.max())

    # slot grids per core; weights carry dinv[col] folded in
    wslab = np.zeros((N_CORES, P, T), dtype=np.float32)
    islab = np.zeros((N_CORES, P, T), dtype=np.int32)
    order = np.lexsort((p_of_edge, b_of_edge, core_of_edge))
    ce, be, pe = core_of_edge[order], b_of_edge[order], p_of_edge[order]
    se = src_row[order]
    we = (ws * dinv_old[cols])[order]
    key = (ce * NB + be) * P + pe
    start = np.r_[True, key[1:] != key[:-1]]
    gidx = np.arange(len(key)) - np.maximum.accumulate(
        np.where(start, np.arange(len(key)), 0))
    colpos = coffs[be] + gidx
    wslab[ce, pe, colpos] = we
    islab[ce, pe, colpos] = se.astype(np.int32)

    # host-side dinv per (core, p, b); dummy slots get dinv = 1 (deg 1)
    dinv_slab = np.ones((N_CORES, P, NB), dtype=np.float32)
    ls = np.arange(NLOC)
    bb, pp_ = ls // P, ls % P
    outmap = []
    xTs = []
    for c in range(N_CORES):
        s_new = (bb * N_CORES + c) * P + pp_
        real = s_new >= ND
        old_ids = np.full(NLOC, -1, dtype=np.int64)
        old_ids[real] = perm[s_new[real] - ND]
        dloc_arr = np.ones(NLOC, dtype=np.float32)
        dloc_arr[real] = dinv_old[old_ids[real]]
        dinv_slab[c] = dloc_arr.reshape(NB, P).T
        Xc = np.zeros((NLOC, F_PAD), dtype=np.float32)
        Xc[real, :F_IN] = x[old_ids[real]]
        xTs.append(np.ascontiguousarray(Xc.T).astype(ml_dtypes.bfloat16))
        outmap.append(old_ids)

    W1p = np.zeros((F_PAD, H), dtype=np.float32)
    W1p[:F_IN] = W1
    W1p = W1p.astype(ml_dtypes.bfloat16)
    in_maps = []
    for c in range(N_CORES):
        in_maps.append({
            "xT": xTs[c], "W1p": W1p, "b1t": np.tile(b1[None, :], (P, 1)),
            "W2t": W2.copy(), "b2t": np.tile(b2[None, :], (P, 1)),
            "wsl": wslab[c].astype(ml_dtypes.bfloat16), "isl": islab[c], "dvt": dinv_slab[c],
        })
    return in_maps, outmap, (T, tuple(d_loc.tolist()), tuple(coffs.tolist()), dmax)


def kernel(x, edge_index, edge_weight, W1, b1, W2, b2):
    from concourse.bass_utils import run_bass_kernel_spmd

    in_maps, outmap, (T, d_loc, coffs, dmax) = _prep(
        x, edge_index, edge_weight, W1, b1, W2, b2)

    key = (T, d_loc, coffs, dmax)
    if key not in _PROG_CACHE:
        _PROG_CACHE[key] = _build_program(T, list(d_loc), list(coffs), dmax)
    nc = _PROG_CACHE[key]

    global LAST_EXEC_NS, LAST_TRACE, LAST_PROFILE_JSON
    res = run_bass_kernel_spmd(nc, in_maps, core_ids=list(range(N_CORES)),
                               trace=PROFILE)
    if res.exec_time_ns:
        LAST_EXEC_NS = res.exec_time_ns
    if res.instructions_and_trace is not None:
        LAST_TRACE = res.instructions_and_trace[1]
    if res.profile_json is not None:
        LAST_PROFILE_JSON = res.profile_json
    out = np.zeros((N_NODES, C), dtype=np.float32)
    for c in range(N_CORES):
        oc = np.asarray(res.results[c]["outd"], dtype=np.float32)
        m = outmap[c]
        real = m >= 0
        out[m[real]] = oc[real]
    return out



# revision 14
# speedup vs baseline: 1.1894x; 1.1894x over previous
"""2-layer GCN (GCNConv -> ReLU -> GCNConv -> log_softmax) on 8 Trainium2 cores.

Baseline (per-slot indirect DMAs) — kept for device sanity checks.
"""

import ml_dtypes
import numpy as np

N_NODES = 100000
N_EDGES = 1600000
F_IN = 500
F_PAD = 512
H = 64
C = 16
N_CORES = 8
P = 128
NPAD = 100352          # 8 * 12544 node slots after padding
ND = NPAD - N_NODES    # dummy slots (placed first: lowest degree)
NB = NPAD // (P * N_CORES)   # 98 local blocks per core
NLOC = NB * P                # 12544 local node slots per core

_PROG_CACHE = {}
PROFILE = False
LAST_EXEC_NS = None
LAST_TRACE = None
LAST_PROFILE_JSON = None


def _build_program(T, d_loc, coffs, dmax):
    import concourse.bacc as bacc
    import concourse.bass as bass
    import concourse.mybir as mybir
    from concourse.tile import TileContext
    from concourse.masks import make_identity
    from contextlib import ExitStack

    dt = mybir.dt.float32
    bf = mybir.dt.bfloat16
    nc = bacc.Bacc("TRN2", target_bir_lowering=False, debug=False,
                   num_devices=N_CORES, dynamic_dma_scratch_size=32768)

    xT = nc.dram_tensor("xT", [F_PAD, NLOC], bf, kind="ExternalInput")
    W1p = nc.dram_tensor("W1p", [F_PAD, H], bf, kind="ExternalInput")
    b1t = nc.dram_tensor("b1t", [P, H], dt, kind="ExternalInput")
    W2t = nc.dram_tensor("W2t", [H, C], dt, kind="ExternalInput")
    b2t = nc.dram_tensor("b2t", [P, C], dt, kind="ExternalInput")
    wsl = nc.dram_tensor("wsl", [P, T], bf, kind="ExternalInput")
    isl = nc.dram_tensor("isl", [P, T], mybir.dt.int32, kind="ExternalInput")
    dvt = nc.dram_tensor("dvt", [P, NB], dt, kind="ExternalInput")
    outd = nc.dram_tensor("outd", [NLOC, C], dt, kind="ExternalOutput")

    # gather tables: dedicated DRAM tensors (offset-0 requirement of
    # indirect_dma_start sources)
    hs1_loc = nc.dram_tensor("hs1_loc", [NLOC, H], bf)
    hs1_full = nc.dram_tensor("hs1_full", [NPAD, H], bf, addr_space="Shared")
    g1_loc = nc.dram_tensor("g1_loc", [NLOC, H], bf)
    g1_full = nc.dram_tensor("g1_full", [NPAD, H], bf, addr_space="Shared")

    SG = 7            # blocks per transform supergroup
    NSG = NB // SG    # 14
    HB = NB // 2
    HROWS = N_CORES * HB * P

    def bcast_inner(ap, n):
        # append a step-0 inner dim of size n to an AP view
        return bass.AP(ap.tensor, ap.offset, list(ap.ap) + [[0, n]])

    def swap_last2(ap):
        a = list(ap.ap)
        a[-1], a[-2] = a[-2], a[-1]
        return bass.AP(ap.tensor, ap.offset, a)

    with TileContext(nc) as tc, ExitStack() as ctx:
        cp = ctx.enter_context(tc.tile_pool(name="const", bufs=1))
        xp = ctx.enter_context(tc.tile_pool(name="xsg", bufs=2))
        hp = ctx.enter_context(tc.tile_pool(name="hrow", bufs=4))
        gp = ctx.enter_context(tc.tile_pool(name="gsg", bufs=2))
        mp = ctx.enter_context(tc.tile_pool(name="msg", bufs=10))
        ap_ = ctx.enter_context(tc.tile_pool(name="agg", bufs=8))
        pp = ctx.enter_context(tc.tile_pool(name="ps", bufs=4, space="PSUM"))
        pt = ctx.enter_context(tc.tile_pool(name="pstr", bufs=2, space="PSUM"))
        po = ctx.enter_context(tc.tile_pool(name="pso", bufs=2, space="PSUM"))
        sp = ctx.enter_context(tc.tile_pool(name="small", bufs=12))

        # ---- constants ----
        w1t = cp.tile([P, 4, H], bf)
        for k in range(4):
            nc.sync.dma_start(w1t[:, k, :], W1p[k * P:(k + 1) * P, :])
        b1r = cp.tile([P, H], dt)
        nc.sync.dma_start(b1r[:], b1t[:])
        w2 = cp.tile([H, C], dt)
        nc.sync.dma_start(w2[:], W2t[:])
        b2r = cp.tile([P, C], dt)
        nc.sync.dma_start(b2r[:], b2t[:])
        wslt = cp.tile([P, T], bf)
        nc.sync.dma_start(wslt[:], wsl[:])
        islt = cp.tile([P, T], mybir.dt.int32)
        nc.sync.dma_start(islt[:], isl[:])
        dinv = cp.tile([P, NB], dt)
        nc.sync.dma_start(dinv[:], dvt[:])
        ident = cp.tile([P, P], dt)
        make_identity(nc, ident[:])
        dsq = cp.tile([P, NB], dt)
        nc.vector.tensor_tensor(out=dsq[:], in0=dinv[:], in1=dinv[:],
                                op=mybir.AluOpType.mult)

        # persistent per-block tiles
        hself = cp.tile([P, NB, H], dt)    # dinv^2 * t1 + b1 (self term + bias)
        g1self = cp.tile([P, NB, H], dt)   # dinv * g1 (layer-2 self term)
        o_all = cp.tile([P, NB, C], dt)
        maxv = cp.tile([P, NB], dt)
        s_all = cp.tile([P, NB], dt)
        lns = cp.tile([P, NB], dt)

        # ---- transform: t1 = x @ W1 ; hs1 = dinv * t1 ----
        for sg in range(NSG):
            c0 = sg * SG * P
            xk = xp.tile([P, 4, SG * P], bf, tag="xk")
            for k in range(4):
                nc.sync.dma_start(xk[:, k, :], xT[k * P:(k + 1) * P, c0:c0 + SG * P])
            hsg = gp.tile([P, SG, H], bf, tag="hsg")
            for bl in range(SG):
                b = sg * SG + bl
                ps = pp.tile([P, H], dt, tag="pst")
                for k in range(4):
                    nc.tensor.matmul(ps[:], lhsT=xk[:, k, bl * P:(bl + 1) * P],
                                     rhs=w1t[:, k, :],
                                     start=(k == 0), stop=(k == 3))
                nc.vector.tensor_scalar(hsg[:, bl, :], ps[:], dinv[:, b:b + 1],
                                        None, op0=mybir.AluOpType.mult)
                nc.vector.scalar_tensor_tensor(
                    out=hself[:, b, :], in0=ps[:],
                    scalar=dsq[:, b:b + 1], in1=b1r[:],
                    op0=mybir.AluOpType.mult, op1=mybir.AluOpType.add)
            for bl in range(SG):
                b = sg * SG + bl
                nc.sync.dma_start(hs1_loc[b * P:(b + 1) * P, :], hsg[:, bl, :])
            if (sg + 1) * SG == HB:
                # first half of the local slice is complete -> overlap its
                # AllGather with the rest of the transform
                nc.gpsimd.collective_compute(
                    "AllGather", mybir.AluOpType.bypass,
                    replica_groups=[list(range(N_CORES))],
                    ins=[hs1_loc[0:HB * P, :]], outs=[hs1_full[0:HROWS, :]])

        nc.gpsimd.collective_compute(
            "AllGather", mybir.AluOpType.bypass,
            replica_groups=[list(range(N_CORES))],
            ins=[hs1_loc[HB * P:NLOC, :]], outs=[hs1_full[HROWS:NPAD, :]])

        # ---- aggregation layers ----
        def agg_layer(table, post, blo, bhi):
            for b in range(blo, bhi):
                db = d_loc[b]
                msg = mp.tile([P, dmax, H], bf, tag="msg")
                for j in range(db):
                    nc.gpsimd.indirect_dma_start(
                        out=msg[:, j, :], out_offset=None, in_=table[:],
                        in_offset=bass.IndirectOffsetOnAxis(
                            ap=islt[:, coffs[b] + j:coffs[b] + j + 1], axis=0))
                # weighted sum over the db in-edge slots (weights include
                # dinv[col])
                wv = bcast_inner(wslt[:, coffs[b]:coffs[b] + db], H)
                nc.vector.tensor_tensor(out=msg[:, :db, :], in0=msg[:, :db, :],
                                        in1=wv, op=mybir.AluOpType.mult)
                agg = ap_.tile([P, H], dt, tag="agg")
                nc.vector.reduce_sum(agg[:], swap_last2(msg[:, :db, :]),
                                     axis=mybir.AxisListType.X)
                post(b, agg)

        # layer 1 post: aggf = agg + (dinv^2 t1 + b1) ; g1 = dinv * relu(aggf)
        g1sg = {}

        def post1(b, agg):
            nc.vector.tensor_tensor(out=agg[:], in0=agg[:],
                                    in1=hself[:, b, :],
                                    op=mybir.AluOpType.add)
            bl = b % SG
            if bl == 0:
                g1t_new = gp.tile([P, SG, H], bf, tag="g1sg")
                g1sg[0] = g1t_new
            g1t = g1sg[0]
            nc.vector.tensor_scalar_max(agg[:], agg[:], 0.0)
            nc.vector.tensor_scalar(g1t[:, bl, :], agg[:], dinv[:, b:b + 1],
                                    None, op0=mybir.AluOpType.mult)
            nc.vector.tensor_scalar(g1self[:, b, :], agg[:],
                                    dsq[:, b:b + 1], None,
                                    op0=mybir.AluOpType.mult)
            nc.sync.dma_start(g1_loc[b * P:(b + 1) * P, :], g1t[:, bl, :])

        agg_layer(hs1_full, post1, 0, HB)
        # first half of g1 done -> overlap its AllGather with the second half
        nc.gpsimd.collective_compute(
            "AllGather", mybir.AluOpType.bypass,
            replica_groups=[list(range(N_CORES))],
            ins=[g1_loc[0:HB * P, :]], outs=[g1_full[0:HROWS, :]])
        agg_layer(hs1_full, post1, HB, NB)
        nc.gpsimd.collective_compute(
            "AllGather", mybir.AluOpType.bypass,
            replica_groups=[list(range(N_CORES))],
            ins=[g1_loc[HB * P:NLOC, :]], outs=[g1_full[HROWS:NPAD, :]])

        # layer 2 post: sc2 = agg2 + dinv*g1 ; o = sc2 @ W2 + b2 (+ row max)
        def post2(b, agg):
            nc.vector.tensor_tensor(out=agg[:], in0=agg[:],
                                    in1=g1self[:, b, :],
                                    op=mybir.AluOpType.add)
            ptr = pt.tile([H, P], dt, tag="ptr")
            nc.tensor.transpose(ptr[:], agg[:], ident[:])
            scT = sp.tile([H, P], dt, tag="scT")
            nc.vector.tensor_copy(scT[:], ptr[:])
            pso = po.tile([P, C], dt, tag="pso")
            nc.tensor.matmul(pso[:], lhsT=scT[:], rhs=w2[:],
                             start=True, stop=True)
            nc.vector.tensor_tensor(out=o_all[:, b, :], in0=pso[:],
                                    in1=b2r[:], op=mybir.AluOpType.add)
            nc.vector.tensor_reduce(maxv[:, b:b + 1], o_all[:, b, :],
                                    axis=mybir.AxisListType.X,
                                    op=mybir.AluOpType.max)

        # ---- batched log_softmax tail (in place on o_all), per half so the
        # first half overlaps the second half's gathers ----
        def softmax_tail(blo, bhi):
            nc.vector.tensor_tensor(out=o_all[:, blo:bhi, :],
                                    in0=o_all[:, blo:bhi, :],
                                    in1=bcast_inner(maxv[:, blo:bhi], C),
                                    op=mybir.AluOpType.subtract)
            for b in range(blo, bhi):
                e = sp.tile([P, C], dt, tag="e")
                nc.scalar.activation(e[:], o_all[:, b, :],
                                     mybir.ActivationFunctionType.Exp,
                                     accum_out=s_all[:, b:b + 1])
            nc.scalar.activation(lns[:, blo:bhi], s_all[:, blo:bhi],
                                 mybir.ActivationFunctionType.Ln)
            nc.vector.tensor_tensor(out=o_all[:, blo:bhi, :],
                                    in0=o_all[:, blo:bhi, :],
                                    in1=bcast_inner(lns[:, blo:bhi], C),
                                    op=mybir.AluOpType.subtract)
            for b in range(blo, bhi):
                nc.sync.dma_start(outd[b * P:(b + 1) * P, :], o_all[:, b, :])

        for blo, bhi in ((0, HB), (HB, 86), (86, NB)):
            agg_layer(g1_full, post2, blo, bhi)
            softmax_tail(blo, bhi)

    nc.compile()
    return nc


def _prep(x, edge_index, edge_weight, W1, b1, W2, b2):
    x = np.asarray(x, dtype=np.float32)
    ei = np.asarray(edge_index).astype(np.int64)
    ew = np.asarray(edge_weight, dtype=np.float32)
    W1 = np.asarray(W1, dtype=np.float32)
    b1 = np.asarray(b1, dtype=np.float32)
    W2 = np.asarray(W2, dtype=np.float32)
    b2 = np.asarray(b2, dtype=np.float32)

    rows, cols, ws = ei[0], ei[1], ew   # real edges only; self loops special-cased

    # degrees include the self loop (weight 1), matching the reference
    indeg = np.bincount(cols, minlength=N_NODES)
    degw = np.bincount(cols, weights=ws.astype(np.float64),
                       minlength=N_NODES).astype(np.float32) + 1.0
    dinv_old = 1.0 / np.sqrt(degw)      # deg > 0 always (self loop)

    perm = np.argsort(indeg, kind="stable")          # old ids, ascending degree
    new_of_old = np.empty(N_NODES, dtype=np.int64)
    new_of_old[perm] = np.arange(N_NODES, dtype=np.int64) + ND

    HB = NB // 2
    HROWS = N_CORES * HB * P

    def table_row_of_new(s):
        kg = s // P
        p = s % P
        c = kg % N_CORES
        b = kg // N_CORES
        lo = c * HB * P + b * P + p
        hi = HROWS + c * (NB - HB) * P + (b - HB) * P + p
        return np.where(b < HB, lo, hi)

    r_new = new_of_old[rows]
    c_new = new_of_old[cols]
    kg = c_new // P
    core_of_edge = kg % N_CORES
    b_of_edge = kg // N_CORES
    p_of_edge = c_new % P
    src_row = table_row_of_new(r_new)

    # per-local-block chunk counts across cores (no self loops)
    cnt_key = ((core_of_edge * P + p_of_edge) * NB + b_of_edge)
    cnt = np.bincount(cnt_key, minlength=N_CORES * P * NB).reshape(
        N_CORES, P, NB)
    d_loc = cnt.max(axis=(0, 1)).astype(np.int64)
    d_loc = np.maximum(d_loc, 1)
    coffs = np.zeros(NB, dtype=np.int64)
    coffs[1:] = np.cumsum(d_loc)[:-1]
    T = int(d_loc.sum())
    dmax = int(d_loc.max())

    # slot grids per core; weights carry dinv[col] folded in
    wslab = np.zeros((N_CORES, P, T), dtype=np.float32)
    islab = np.zeros((N_CORES, P, T), dtype=np.int32)
    order = np.lexsort((p_of_edge, b_of_edge, core_of_edge))
    ce, be, pe = core_of_edge[order], b_of_edge[order], p_of_edge[order]
    se = src_row[order]
    we = (ws * dinv_old[cols])[order]
    key = (ce * NB + be) * P + pe
    start = np.r_[True, key[1:] != key[:-1]]
    gidx = np.arange(len(key)) - np.maximum.accumulate(
        np.where(start, np.arange(len(key)), 0))
    colpos = coffs[be] + gidx
    wslab[ce, pe, colpos] = we
    islab[ce, pe, colpos] = se.astype(np.int32)

    # host-side dinv per (core, p, b); dummy slots get dinv = 1 (deg 1)
    dinv_slab = np.ones((N_CORES, P, NB), dtype=np.float32)
    ls = np.arange(NLOC)
    bb, pp_ = ls // P, ls % P
    outmap = []
    xTs = []
    for c in range(N_CORES):
        s_new = (bb * N_CORES + c) * P + pp_
        real = s_new >= ND
        old_ids = np.full(NLOC, -1, dtype=np.int64)
        old_ids[real] = perm[s_new[real] - ND]
        dloc_arr = np.ones(NLOC, dtype=np.float32)
        dloc_arr[real] = dinv_old[old_ids[real]]
        dinv_slab[c] = dloc_arr.reshape(NB, P).T
        Xc = np.zeros((NLOC, F_PAD), dtype=np.float32)
        Xc[real, :F_IN] = x[old_ids[real]]
        xTs.append(np.ascontiguousarray(Xc.T).astype(ml_dtypes.bfloat16))
        outmap.append(old_ids)

    W1p = np.zeros((F_PAD, H), dtype=np.float32)
    W1p[:F_IN] = W1
    W1p = W1p.astype(ml_dtypes.bfloat16)
    in_maps = []
    for c in range(N_CORES):
        in_maps.append({
            "xT": xTs[c], "W1p": W1p, "b1t": np.tile(b1[None, :], (P, 1)),
            "W2t": W2.copy(), "b2t": np.tile(b2[None, :], (P, 1)),
            "wsl": wslab[c].astype(ml_dtypes.bfloat16), "isl": islab[c], "dvt": dinv_slab[c],
        })
    return in_maps, outmap, (T, tuple(d_loc.tolist()), tuple(coffs.tolist()), dmax)


def kernel(x, edge_index, edge_weight, W1, b1, W2, b2):
    from concourse.bass_utils import run_bass_kernel_spmd

    in_maps, outmap, (T, d_loc, coffs, dmax) = _prep(
        x, edge_index, edge_weight, W1, b1, W2, b2)

    key = (T, d_loc, coffs, dmax)
    if key not in _PROG_CACHE:
        _PROG_CACHE[key] = _build_program(T, list(d_loc), list(coffs), dmax)
    nc = _PROG_CACHE[key]

    global LAST_EXEC_NS, LAST_TRACE, LAST_PROFILE_JSON
    res = run_bass_kernel_spmd(nc, in_maps, core_ids=list(range(N_CORES)),
                               trace=PROFILE)
    if res.exec_time_ns:
        LAST_EXEC_NS = res.exec_time_ns
    if res.instructions_and_trace is not None:
        LAST_TRACE = res.instructions_and_trace[1]
    if res.profile_json is not None:
        LAST_PROFILE_JSON = res.profile_json
    out = np.zeros((N_NODES, C), dtype=np.float32)
    for c in range(N_CORES):
        oc = np.asarray(res.results[c]["outd"], dtype=np.float32)
        m = outmap[c]
        real = m >= 0
        out[m[real]] = oc[real]
    return out
